# revision 3
# baseline (speedup 1.0000x reference)
"""Binarized linear: out = sign(x+eps) @ sign(w+eps).T on 8 trn2 cores.

Sharding: 4x2 grid. Core c=(r,s): rows x[r*2048:(r+1)*2048], rows w[s*2048:(s+1)*2048].
Each core computes a [2048, 2048] output block; host concatenates. No collectives.

Per-core kernel (all arithmetic exact -> rel err 0 vs the f32 reference):
  - binarize both operands to bf16 as +/-0.5 on DVE ((v>=0)-0.5, matching
    sign(v+1e-20) for f32 randn inputs)
  - transpose to K-on-partition layout with the DMA XBAR (dma_start_transpose,
    2-byte dtype): one instruction turns a [128, 2048] bf16 half-block into
    [128, 16, 128] k-tiles. x-transposes ride the SP queue, w-transposes the
    ACT queue, so the PE does no transpose work at all.
  - cast bf16 -> fp8e4 on ACT into the resident xbT/wbT operands
  - fp8 DoubleRow matmuls accumulate K=256 per instruction into fp32 PSUM
    (products +/-0.25, sums exact multiples of 0.25 well under 2^24)
  - gpsimd evicts PSUM with a x4 scale -> exact integer outputs

The PE stream is 1024 DR matmuls (16 ib x 4 jc x 16 kp) of ~231 ns: that is
the fp8 roofline for this shape. Startup warmup matmuls on an identity keep
the PE p-state ramping while the first inputs land.
"""

import numpy as np

P = 128
GRID_I, GRID_J = 4, 2
N_CORES = 8
FULL_M, FULL_N, FULL_K = 8192, 4096, 4096
M_SH, N_SH = FULL_M // GRID_I, FULL_N // GRID_J  # 2048, 2048

_PROGRAM_CACHE = {}


def build_program(m_sh=M_SH, n_sh=N_SH, k=FULL_K, warmup=48):
    """Build (and cache) the per-core Bass program. Same SPMD program on all cores."""
    key = (m_sh, n_sh, k, warmup)
    if key in _PROGRAM_CACHE:
        return _PROGRAM_CACHE[key]

    from contextlib import ExitStack

    import concourse.bass as bass
    import concourse.mybir as mybir
    from concourse import bacc, tile
    from concourse.masks import make_identity

    f32 = mybir.dt.float32
    bf16 = mybir.dt.bfloat16
    fp8 = mybir.dt.float8e4

    KT = k // P          # 32 k tiles of 128
    KH = k // 2          # 2048: half-row staging width
    HT = KT // 2         # 16 k-tiles per half
    IB = m_sh // P       # 16 x blocks
    JB = n_sh // P       # 16 w blocks
    JBLK = 512           # j chunk width (matmul free dim)
    JC = n_sh // JBLK    # 4 j chunks
    JB_PER_JC = JBLK // P  # 4 w blocks per chunk

    nc = bacc.Bacc("TRN2", target_bir_lowering=False, debug=False)
    xs = nc.dram_tensor("xs", [m_sh, k], f32, kind="ExternalInput").ap()
    ws = nc.dram_tensor("ws", [n_sh, k], f32, kind="ExternalInput").ap()
    out = nc.dram_tensor("out", [m_sh, n_sh], f32, kind="ExternalOutput").ap()

    with tile.TileContext(nc) as tc, ExitStack() as ctx:
        const_pool = ctx.enter_context(tc.tile_pool(name="const", bufs=1))
        stage_x = ctx.enter_context(tc.tile_pool(name="stagex", bufs=3))
        stage_w = ctx.enter_context(tc.tile_pool(name="stagew", bufs=2))
        b16_pool = ctx.enter_context(tc.tile_pool(name="b16", bufs=4))
        t16_pool = ctx.enter_context(tc.tile_pool(name="t16", bufs=4))
        xbt_pool = ctx.enter_context(tc.tile_pool(name="xbt", bufs=1))
        wbt_pool = ctx.enter_context(tc.tile_pool(name="wbt", bufs=1))
        out_pool = ctx.enter_context(tc.tile_pool(name="outp", bufs=3))
        psum_mm = ctx.enter_context(tc.tile_pool(name="psmm", bufs=4, space="PSUM"))

        ident = const_pool.tile([P, P], fp8, tag="ident")
        make_identity(nc, ident)

        # PE warmup: matmuls with no data dependency so the clock-gate ramps
        # while the first input DMAs are still in flight.
        psum_wu = ctx.enter_context(tc.tile_pool(name="pswu", bufs=1, space="PSUM"))
        warm_psum = psum_wu.tile([P, P], f32, tag="warm", name="warm") if warmup else None

        def warm(n):
            for _ in range(n):
                nc.tensor.matmul(warm_psum[:], lhsT=ident[:], rhs=ident[:],
                                 start=True, stop=True)

        if warmup:
            warm(warmup)

        # Resident transposed binarized operands, K on partitions:
        #   xbT[ib][kp, kt, i] = bin(xs[ib*128 + i, kt*128 + kp])   (+/-0.5)
        #   wbT[jc][kp, kt, j] = bin(ws[jc*512 + j, kt*128 + kp])   (+/-0.5)
        xbT = [
            xbt_pool.tile([P, KT, P], fp8, tag=f"xbt{ib}", name=f"xbt{ib}")
            for ib in range(IB)
        ]
        wbT = [
            wbt_pool.tile([P, KT, JBLK], fp8, tag=f"wbt{jc}", name=f"wbt{jc}")
            for jc in range(JC)
        ]

        def prep_half(src, blk, h, stage_pool, stg_tag, is_x):
            """One [128, 2048] half-block: load f32, binarize to bf16 +/-0.5,
            DMA-XBAR transpose to [128, 16, 128], cast to fp8 into residents."""
            stg = stage_pool.tile([P, KH], f32, tag=stg_tag, name=stg_tag)
            nc.sync.dma_start(stg[:], src[blk * P:(blk + 1) * P, h * KH:(h + 1) * KH])
            b16 = b16_pool.tile([P, KH], bf16, tag="b16", name="b16")
            nc.vector.tensor_scalar(
                b16[:], stg[:], 0.0, 0.5,
                mybir.AluOpType.is_ge, mybir.AluOpType.subtract,
            )
            t16 = t16_pool.tile([P, HT, P], bf16, tag="t16", name="t16")
            if is_x:
                nc.sync.dma_start_transpose(t16[:], b16[:])
                nc.scalar.copy(xbT[blk][:, h * HT:(h + 1) * HT, :], t16[:])
            else:
                nc.scalar.dma_start_transpose(t16[:], b16[:])
                jc, sub = divmod(blk, JB_PER_JC)
                nc.scalar.copy(
                    wbT[jc][:, h * HT:(h + 1) * HT, sub * P:(sub + 1) * P], t16[:]
                )

        def prep_x(ib):
            prep_half(xs, ib, 0, stage_x, "stgx", True)
            prep_half(xs, ib, 1, stage_x, "stgx", True)

        # Later w chunks stream in at half-block granularity so their DMA
        # bursts never displace the critical x-prep chain.
        w_state = {"jb": JB_PER_JC, "half": 0}

        def prep_w_half():
            jb = w_state["jb"]
            if jb >= JB:
                return
            prep_half(ws, jb, w_state["half"], stage_w, "stgw", False)
            if w_state["half"] == 1:
                w_state["jb"] += 1
                w_state["half"] = 0
            else:
                w_state["half"] = 1

        def mm_block(ib, jc):
            ps = psum_mm.tile([P, JBLK], f32, tag="ps", name="ps")
            nk = KT // 2
            for kp in range(nk):
                nc.tensor.matmul(
                    ps[:],
                    lhsT=xbT[ib][:, 2 * kp:2 * kp + 2, :],
                    rhs=wbT[jc][:, 2 * kp:2 * kp + 2, :],
                    start=(kp == 0), stop=(kp == nk - 1),
                    perf_mode=mybir.MatmulPerfMode.DoubleRow,
                )
            ob = out_pool.tile([P, JBLK], f32, tag="ob", name="ob")
            # products are +/-0.25 -> scale by 4 (gpsimd cannot read PSUM)
            nc.vector.tensor_scalar_mul(ob[:], ps[:], 4.0)
            nc.sync.dma_start(
                out[ib * P:(ib + 1) * P, jc * JBLK:(jc + 1) * JBLK], ob[:]
            )

        # Startup: land the operands of mm_block(0, 0) in k-order (h=0 of w
        # chunk 0 and x block 0 first), padding the PE stream with warmups.
        for jb in range(JB_PER_JC):
            prep_half(ws, jb, 0, stage_w, "stgw", False)
            if warmup:
                warm(10)
        prep_half(xs, 0, 0, stage_x, "stgx", True)
        if warmup:
            warm(10)
        for jb in range(JB_PER_JC):
            prep_half(ws, jb, 1, stage_w, "stgw", False)
            if warmup:
                warm(8)
        prep_half(xs, 0, 1, stage_x, "stgx", True)
        prep_x(1)
        if warmup:
            warm(16)

        # Pass jc over all i blocks; x-prep runs two iterations ahead and the
        # NEXT pass's w chunk is spread across this pass, one half-block every
        # other iteration, keeping input DMA flowing the whole schedule.
        for jc in range(JC):
            for ib in range(IB):
                if jc == 0 and ib + 2 < IB:
                    prep_x(ib + 2)
                if jc < JC - 1 and ib % 2 == 0:
                    prep_w_half()
                mm_block(ib, jc)
        while w_state["jb"] < JB:
            prep_w_half()

    nc.compile()
    _PROGRAM_CACHE[key] = nc
    return nc


def kernel(x, weight):
    x = np.ascontiguousarray(np.asarray(x), dtype=np.float32)
    w = np.ascontiguousarray(np.asarray(weight), dtype=np.float32)
    assert x.shape == (FULL_M, FULL_K) and w.shape == (FULL_N, FULL_K)

    from concourse.bass_utils import run_bass_kernel_spmd

    nc = build_program()
    in_maps = []
    for c in range(N_CORES):
        r, s = divmod(c, GRID_J)
        in_maps.append({
            "xs": x[r * M_SH:(r + 1) * M_SH],
            "ws": w[s * N_SH:(s + 1) * N_SH],
        })
    res = run_bass_kernel_spmd(nc, in_maps, core_ids=list(range(N_CORES))).results
    outp = np.empty((FULL_M, FULL_N), dtype=np.float32)
    for c in range(N_CORES):
        r, s = divmod(c, GRID_J)
        outp[r * M_SH:(r + 1) * M_SH, s * N_SH:(s + 1) * N_SH] = res[c]["out"]
    return outp


# revision 14
# speedup vs baseline: 1.0822x; 1.0822x over previous
"""Binarized linear: out = sign(x+eps) @ sign(w+eps).T on 8 trn2 cores.

Sharding: 4x2 grid. Core c=(r,s): rows x[r*2048:(r+1)*2048], rows w[s*2048:(s+1)*2048].
Each core computes a [2048, 2048] output block; host concatenates. No collectives.

Per-core kernel (all arithmetic exact -> rel err 0 vs the f32 reference):
  - both operands are binarized to fp8e4 +/-0.5 on DVE ((v>=0)-0.5, matching
    sign(v+1e-20) for f32 randn inputs)
  - both operands are transposed to K-on-partition layout with the DMA XBAR
    on the fp8 data viewed as u16 pairs: one dma_start_transpose turns a
    [128, 2048] fp8 half-block into resident tiles
    t[p, kt, 2i+b] = bin[i, 2*(kt*128+p)+b], i.e. partition p carries the
    k-pair (2p, 2p+1) of each 256-wide k-tile. No PE transposes, no casts.
  - fp8 DoubleRow matmuls contract one 256-wide k-tile per instruction: the
    moving operand uses the strided AP [p, b, n] (pairs at stride 1), the
    stationary side uses DoubleRowSwInterleave with [p, m, b], which consumes
    the interleaved layout natively but reverses the stationary rows; the
    host gather flips each 128-row block back (verified exact on hardware).
  - PSUM f32 accumulation is exact (products +/-0.25), DVE evicts with a x4
    scale -> exact integer outputs.

Scheduling: loads stream on the SP queue, transposes on the ACT queue (the
SP-queue XBAR path was observed slower and racy), binarizes and evicts on
DVE, output stores on the gpsimd SWDGE. Each prep chain is software-pipelined
with the loads running a few half-blocks ahead.  mm_blocks
force-advance the chains they consume, so data ordering is correct by
construction.  Two staggered j-chunk passes, (t,0)+(t-4,1) then (t,2)+(t-4,3),
halve the per-PE-second prep demand vs single-chunk passes.  The PE stream is
1024 DR matmuls of ~217 ns issue: the fp8 roofline for this shape.
"""

from collections import deque

import numpy as np

P = 128
GRID_I, GRID_J = 4, 2
N_CORES = 8
FULL_M, FULL_N, FULL_K = 8192, 4096, 4096
M_SH, N_SH = FULL_M // GRID_I, FULL_N // GRID_J  # 2048, 2048

_PROGRAM_CACHE = {}


def build_program(m_sh=M_SH, n_sh=N_SH, k=FULL_K, warmup=True):
    """Build (and cache) the per-core Bass program. Same SPMD program on all cores."""
    key = (m_sh, n_sh, k, warmup)
    if key in _PROGRAM_CACHE:
        return _PROGRAM_CACHE[key]

    from contextlib import ExitStack

    import concourse.bass as bass
    import concourse.mybir as mybir
    from concourse import bacc, tile
    from concourse.masks import make_identity

    f32 = mybir.dt.float32
    fp8 = mybir.dt.float8e4
    u16 = mybir.dt.uint16

    KP = 256             # k per DR instruction (u16 pair tile)
    KT = k // KP         # 16 k-tiles
    KH = k // 2          # 2048: half-row staging width
    HT = KT // 2         # 8 k-tiles per half
    IB = m_sh // P       # 16 x blocks
    JB = n_sh // P       # 16 w blocks
    JBLK = 512           # j chunk width (matmul free dim)
    JC = n_sh // JBLK    # 4 j chunks
    JB_PER_JC = JBLK // P  # 4 w blocks per chunk

    nc = bacc.Bacc("TRN2", target_bir_lowering=False, debug=False)
    xs = nc.dram_tensor("xs", [m_sh, k], f32, kind="ExternalInput").ap()
    ws = nc.dram_tensor("ws", [n_sh, k], f32, kind="ExternalInput").ap()
    out = nc.dram_tensor("out", [m_sh, n_sh], f32, kind="ExternalOutput").ap()

    with tile.TileContext(nc) as tc, ExitStack() as ctx:
        const_pool = ctx.enter_context(tc.tile_pool(name="const", bufs=1))
        stage_x = ctx.enter_context(tc.tile_pool(name="stagex", bufs=4))
        stage_w = ctx.enter_context(tc.tile_pool(name="stagew", bufs=3))
        b8_pool = ctx.enter_context(tc.tile_pool(name="b8", bufs=6))
        xbt_pool = ctx.enter_context(tc.tile_pool(name="xbt", bufs=1))
        wbt_pool = ctx.enter_context(tc.tile_pool(name="wbt", bufs=1))
        out_pool = ctx.enter_context(tc.tile_pool(name="outp", bufs=4))
        psum_mm = ctx.enter_context(tc.tile_pool(name="psmm", bufs=7, space="PSUM"))

        ident = const_pool.tile([P, P], fp8, tag="ident")
        make_identity(nc, ident)

        # PE warmup: matmuls with no data dependency so the clock-gate ramps
        # while the first input DMAs are still in flight.
        psum_wu = ctx.enter_context(tc.tile_pool(name="pswu", bufs=1, space="PSUM"))
        warm_psum = psum_wu.tile([P, P], f32, tag="warm", name="warm") if warmup else None

        def warm(n):
            if not warmup:
                return
            for _ in range(n):
                nc.tensor.matmul(warm_psum[:], lhsT=ident[:], rhs=ident[:],
                                 start=True, stop=True)

        # Resident pair-transposed binarized operands, K on partitions:
        #   xbT[ib][kp, kt, 2i+b] = bin(xs[ib*128 + i, 2*(kt*128+kp) + b])
        #   wbT[jc][kp, kt, 2j+b] = bin(ws[jc*512 + j, 2*(kt*128+kp) + b])
        xbT = [
            xbt_pool.tile([P, KT, 2 * P], fp8, tag=f"xbt{ib}", name=f"xbt{ib}")
            for ib in range(IB)
        ]
        wbT = [
            wbt_pool.tile([P, KT, 2 * JBLK], fp8, tag=f"wbt{jc}", name=f"wbt{jc}")
            for jc in range(JC)
        ]

        class Chain:
            """Software-pipelined load -> binarize -> pair-transpose stream of
            half-blocks. Loads run `lead` items ahead of the transposes; every
            stage of item i is emitted before the transpose of item i."""

            def __init__(self, items, load_fn, bin_fn, t_fn, lead):
                self.items = items
                self.load_fn, self.bin_fn, self.t_fn = load_fn, bin_fn, t_fn
                self.lead = lead
                self.nl = self.nb = self.nt = 0
                self.staged = deque()  # (item, stg) loaded, not yet binarized
                self.binned = deque()  # (item, b8) binarized, not yet transposed

            def _load(self):
                if self.nl < len(self.items):
                    self.staged.append(
                        (self.items[self.nl], self.load_fn(self.items[self.nl]))
                    )
                    self.nl += 1

            def _bin(self):
                if self.nb < self.nl:
                    item, stg = self.staged.popleft()
                    self.binned.append((item, self.bin_fn(item, stg)))
                    self.nb += 1
                elif self.nb < len(self.items):
                    self._load()
                    self._bin()

            def _t(self):
                if self.nt >= self.nb:
                    self._bin()
                if self.nt < self.nb:
                    item, b8 = self.binned.popleft()
                    self.t_fn(item, b8)
                    self.nt += 1

            def advance_to(self, t_idx):
                """Emit transposes through item t_idx (inclusive), keeping
                loads `lead` items ahead."""
                t_idx = min(t_idx, len(self.items) - 1)
                while self.nt <= t_idx:
                    while self.nl < min(len(self.items), self.nt + 1 + self.lead):
                        self._load()
                    self._t()
                while self.nl < min(len(self.items), self.nt + self.lead):
                    self._load()

        def load_half(src, stage_pool, stg_tag, blk, h):
            stg = stage_pool.tile([P, KH], f32, tag=stg_tag, name=stg_tag)
            nc.sync.dma_start(stg[:], src[blk * P:(blk + 1) * P, h * KH:(h + 1) * KH])
            return stg

        def x_load(item):
            blk, h = item
            return load_half(xs, stage_x, "stgx", blk, h)

        def x_bin(item, stg):
            b8 = b8_pool.tile([P, KH], fp8, tag="b8", name="b8")
            nc.vector.tensor_scalar(
                b8[:], stg[:], 0.0, 0.5,
                mybir.AluOpType.is_ge, mybir.AluOpType.subtract,
            )
            return b8

        def x_t(item, b8):
            blk, h = item
            dst = xbT[blk][:, h * HT:(h + 1) * HT, :]
            nc.scalar.dma_start_transpose(dst.bitcast(u16), b8[:].bitcast(u16))

        def w_load(item):
            blk, h = item
            return load_half(ws, stage_w, "stgw", blk, h)

        w_bin = x_bin

        def w_t(item, b8):
            blk, h = item
            jc, sub = divmod(blk, JB_PER_JC)
            dst = wbT[jc][:, h * HT:(h + 1) * HT, sub * 2 * P:(sub + 1) * 2 * P]
            nc.scalar.dma_start_transpose(dst.bitcast(u16), b8[:].bitcast(u16))

        # x halves block-major; w halves chunk-major with h=0 of all four
        # blocks first so the first k-tiles of a chunk land earliest.
        x_items = [(ib, h) for ib in range(IB) for h in range(2)]
        w_items = [(jc * JB_PER_JC + jb, h)
                   for jc in range(JC) for h in range(2) for jb in range(JB_PER_JC)]
        xch = Chain(x_items, x_load, x_bin, x_t, lead=3)
        wch = Chain(w_items, w_load, w_bin, w_t, lead=2)

        def x_done(ib):       # last x item index for block ib
            return 2 * ib + 1

        def w_done(jc):       # last w item index for chunk jc
            return 8 * jc + 7

        # Evicts are emitted five blocks late so the in-order DVE queue
        # reaches them only after their accumulation group has finished.
        pending_ev = deque()

        def flush_ev(n=None):
            for _ in range(len(pending_ev) if n is None else n):
                if not pending_ev:
                    return
                ps, ib, jc = pending_ev.popleft()
                ob = out_pool.tile([P, JBLK], f32, tag="ob", name="ob")
                # products are +/-0.25 -> scale by 4. Stores ride the SP
                # queue; the ACT queue stays a pure transpose stream.
                nc.vector.tensor_scalar_mul(ob[:], ps[:], 4.0)
                nc.sync.dma_start(
                    out[ib * P:(ib + 1) * P, jc * JBLK:(jc + 1) * JBLK], ob[:]
                )

        DRS = mybir.MatmulPerfMode.DoubleRowSwInterleave

        def mm_block(ib, jc):
            xch.advance_to(x_done(ib))
            wch.advance_to(w_done(jc))
            ps = psum_mm.tile([P, JBLK], f32, tag="ps", name="ps")
            for kt in range(KT):
                nc.tensor.matmul(
                    ps[:],
                    lhsT=xbT[ib][:, kt, :].rearrange("p (m b) -> p m b", b=2),
                    rhs=wbT[jc][:, kt, :].rearrange("p (n b) -> p b n", b=2),
                    start=(kt == 0), stop=(kt == KT - 1),
                    perf_mode=DRS,
                )
            pending_ev.append((ps, ib, jc))
            # a 5-block lag: by the time the in-order DVE queue reaches an
            # evict, its accumulation finished long ago, so the evict never
            # head-of-line blocks the binarizes that feed the PE (that
            # coupling locksteps the whole pipeline at ~8.5us/half-block)
            flush_ev(len(pending_ev) - 5)

        # Startup: chunk 0 h=0 and x block 0 land first, warmups cover the PE.
        warm(48)
        wch.advance_to(3)     # w chunk 0, h=0 of all four blocks
        warm(24)
        xch.advance_to(0)     # x block 0 h=0
        warm(24)
        wch.advance_to(7)     # rest of chunk 0
        warm(24)
        xch.advance_to(3)     # x blocks 0-1
        warm(24)

        # Two staggered j-chunk passes: (t, 0) with (t-4, 1), then (t, 2)
        # with (t-4, 3).
        LAG = JB_PER_JC
        for t in range(IB + LAG):
            if t < IB:
                if t + 2 < IB:
                    xch.advance_to(x_done(t + 2))
                # stream w chunks 1-3: chunk 1 over t=0..3, chunks 2-3 spread
                # across the rest of pass A
                wch.advance_to(w_done(1) if t < LAG else
                               min(w_done(3), w_done(1) + 2 * (t - LAG + 1)))
                mm_block(t, 0)
            if t >= LAG:
                mm_block(t - LAG, 1)
        for t in range(IB + LAG):
            if t < IB:
                mm_block(t, 2)
            if t >= LAG:
                mm_block(t - LAG, 3)
        flush_ev()

    nc.compile()
    _PROGRAM_CACHE[key] = nc
    return nc


def kernel(x, weight):
    x = np.ascontiguousarray(np.asarray(x), dtype=np.float32)
    w = np.ascontiguousarray(np.asarray(weight), dtype=np.float32)
    assert x.shape == (FULL_M, FULL_K) and w.shape == (FULL_N, FULL_K)

    from concourse.bass_utils import run_bass_kernel_spmd

    nc = build_program()
    in_maps = []
    for c in range(N_CORES):
        r, s = divmod(c, GRID_J)
        in_maps.append({
            "xs": x[r * M_SH:(r + 1) * M_SH],
            "ws": w[s * N_SH:(s + 1) * N_SH],
        })
    res = run_bass_kernel_spmd(nc, in_maps, core_ids=list(range(N_CORES))).results
    outp = np.empty((FULL_M, FULL_N), dtype=np.float32)
    for c in range(N_CORES):
        r, s = divmod(c, GRID_J)
        # SwInterleave reverses the stationary rows within each 128-block;
        # undo it during the gather.
        blk = res[c]["out"].reshape(M_SH // P, P, N_SH)[:, ::-1, :].reshape(M_SH, N_SH)
        outp[r * M_SH:(r + 1) * M_SH, s * N_SH:(s + 1) * N_SH] = blk
    return outp


# revision 18
# speedup vs baseline: 1.7658x; 1.6317x over previous
"""Binarized linear: out = sign(x+eps) @ sign(w+eps).T on 8 trn2 cores.

Sharding: 4x2 grid. Core c=(r,s): rows x[r*2048:(r+1)*2048], rows w[s*2048:(s+1)*2048].
Each core computes a [2048, 2048] output block; host concatenates. No collectives.

Per-core kernel (all arithmetic exact -> rel err 0 vs the f32 reference):
  - binarize x-shard to fp8e4m3 as +/-0.5 (DVE: (x>=0)-0.5), w-shard as +/-1
    (ACT Sign with +1e-20 bias, matching sign(v+1e-20))
  - transpose both to [K-on-partition] layout via plain fp8 matmul against an
    identity matrix (PE; counts as PE-busy so the HAM clock gate stays warm)
  - fp8 DoubleRow matmuls accumulate K=256 per instruction into fp32 PSUM
    (products +/-0.5, sums exact multiples of 0.5 well under 2^24)
  - evict scales by 2 -> exact integer outputs

Schedule: transpose work is chopped into 4-matmul groups and pumped between
DoubleRow matmuls so the PE never idles; x-preps run two iterations ahead of
their matmul group; later w chunks stream in during earlier passes; warmup
matmuls on the identity pad the startup so the clock gate opens early.
"""

from collections import deque

import numpy as np

P = 128
GRID_I, GRID_J = 4, 2
N_CORES = 8
FULL_M, FULL_N, FULL_K = 8192, 4096, 4096
M_SH, N_SH = FULL_M // GRID_I, FULL_N // GRID_J  # 2048, 2048

_PROGRAM_CACHE = {}


def build_program(m_sh=M_SH, n_sh=N_SH, k=FULL_K, use_dr=True, warmup=64,
                  interleave=True):
    """Build (and cache) the per-core Bass program. Same SPMD program on all cores."""
    key = (m_sh, n_sh, k, use_dr, warmup, interleave)
    if key in _PROGRAM_CACHE:
        return _PROGRAM_CACHE[key]

    from contextlib import ExitStack

    import concourse.bass as bass
    import concourse.mybir as mybir
    from concourse import bacc, tile
    from concourse.masks import make_identity

    f32 = mybir.dt.float32
    fp8 = mybir.dt.float8e4

    KT = k // P          # number of 128-wide k tiles
    KH = k // 2          # half-row staging width
    IB = m_sh // P       # i blocks (x rows / 128)
    JB = n_sh // P       # j blocks (w rows / 128)
    JBLK = 512           # j chunk width (matmul free dim)
    JC = n_sh // JBLK    # j chunks
    JB_PER_JC = JBLK // P
    KG = 4               # k-tiles per transpose-evict group
    assert KT % KG == 0 and KT % 2 == 0

    nc = bacc.Bacc("TRN2", target_bir_lowering=False, debug=False)
    xs = nc.dram_tensor("xs", [m_sh, k], f32, kind="ExternalInput").ap()
    ws = nc.dram_tensor("ws", [n_sh, k], f32, kind="ExternalInput").ap()
    out = nc.dram_tensor("out", [m_sh, n_sh], f32, kind="ExternalOutput").ap()

    with tile.TileContext(nc) as tc, ExitStack() as ctx:
        const_pool = ctx.enter_context(tc.tile_pool(name="const", bufs=1))
        stage_x = ctx.enter_context(tc.tile_pool(name="stagex", bufs=4))
        stage_w = ctx.enter_context(tc.tile_pool(name="stagew", bufs=2))
        b8_pool = ctx.enter_context(tc.tile_pool(name="b8", bufs=3))
        xbt_pool = ctx.enter_context(tc.tile_pool(name="xbt", bufs=1))
        wbt_pool = ctx.enter_context(tc.tile_pool(name="wbt", bufs=1))
        out_pool = ctx.enter_context(tc.tile_pool(name="outp", bufs=3))
        psum_t = ctx.enter_context(tc.tile_pool(name="pst", bufs=4, space="PSUM"))
        psum_mm = ctx.enter_context(tc.tile_pool(name="psmm", bufs=3, space="PSUM"))

        ident = const_pool.tile([P, P], fp8, tag="ident")
        make_identity(nc, ident)
        sign_bias = const_pool.tile([P, 1], f32, tag="sbias")
        nc.any.memset(sign_bias[:], 1e-20)

        # PE warmup: matmuls with no data dependency so the HAM clock-gate
        # opens to 8/8 while the first input DMAs are still in flight.
        psum_wu = ctx.enter_context(tc.tile_pool(name="pswu", bufs=1, space="PSUM"))
        warm_psum = psum_wu.tile([P, P], f32, tag="warm", name="warm") if warmup else None

        def warm(n):
            for _ in range(n):
                nc.tensor.matmul(warm_psum[:], lhsT=ident[:], rhs=ident[:],
                                 start=True, stop=True)

        if warmup:
            warm(warmup)

        # Resident transposed binarized operands, K on partitions:
        #   xbT[ib][kp, kt, i] = bin(xs[ib*128 + i, kt*128 + kp])   (+/-0.5)
        #   wbT[jc][kp, kt, j] = bin(ws[jc*512 + j, kt*128 + kp])   (+/-1)
        xbT = [
            xbt_pool.tile([P, KT, P], fp8, tag=f"xbt{ib}", name=f"xbt{ib}")
            for ib in range(IB)
        ]
        wbT = [
            wbt_pool.tile([P, KT, JBLK], fp8, tag=f"wbt{jc}", name=f"wbt{jc}")
            for jc in range(JC)
        ]

        def bin_x(b8h, stgh):
            # (v >= 0) -> {1,0}; minus 0.5 -> +/-0.5. Matches sign(v+1e-20) up
            # to the measure-zero region (-1e-20, 0) that f32 randn never hits.
            nc.vector.tensor_scalar(
                b8h, stgh, 0.0, 0.5,
                mybir.AluOpType.is_ge, mybir.AluOpType.subtract,
            )

        def bin_w(b8h, stgh):
            nc.scalar.sign(b8h, stgh, bias=sign_bias[:])  # sign(w+1e-20) -> +/-1

        def load_binarize(src_rows, pool, stg_tag, b8_tag, binarize, eng=None):
            """Load 128 rows x k f32 (two half DMAs), binarize to fp8."""
            eng = eng or nc.sync
            b8 = b8_pool.tile([P, k], fp8, tag=b8_tag, name=b8_tag)
            for h in range(2):
                stg = pool.tile([P, KH], f32, tag=stg_tag, name=stg_tag)
                eng.dma_start(stg[:], src_rows[:, h * KH:(h + 1) * KH])
                binarize(b8[:, h * KH:(h + 1) * KH], stg[:])
            return b8

        pending = deque()  # transpose-group closures (each ~4 PE matmuls)

        def queue_tgroups(b8, dest, dest_col0, which):
            for kg in range(KT // KG):
                def g(kg=kg, b8=b8, dest=dest, dest_col0=dest_col0):
                    pt = psum_t.tile([P, KG, P], f32, tag="pt", name="pt")
                    for t in range(KG):
                        kt = kg * KG + t
                        nc.tensor.matmul(
                            pt[:, t, :],
                            lhsT=b8[:, kt * P:(kt + 1) * P],
                            rhs=ident[:],
                            start=True, stop=True,
                        )
                    nc.any.tensor_copy(
                        dest[:, kg * KG:(kg + 1) * KG, dest_col0:dest_col0 + P],
                        pt[:],
                    )
                pending.append(g)

        def pump(n):
            for _ in range(n):
                if not pending:
                    return
                pending.popleft()()

        def prep_x(ib):
            b8 = load_binarize(xs[ib * P:(ib + 1) * P, :], stage_x, "stgx", "xb8",
                               bin_x)
            queue_tgroups(b8, xbT[ib], 0, "x")

        def prep_w(jb):
            jc, sub = divmod(jb, JB_PER_JC)
            b8 = load_binarize(ws[jb * P:(jb + 1) * P, :], stage_w, "stgw", "wb8",
                               bin_w, eng=nc.scalar)
            queue_tgroups(b8, wbT[jc], sub * P, "w")

        # Later w chunks stream in at half-block granularity (one 1MB load +
        # one Sign every other iteration) so their DMA/ACT bursts never
        # displace the critical x-prep chain.
        w_state = {"jb": JB_PER_JC, "half": 0, "b8": None}

        def prep_w_half():
            jb = w_state["jb"]
            if jb >= JB:
                return
            h = w_state["half"]
            if h == 0:
                w_state["b8"] = b8_pool.tile([P, k], fp8, tag="wb8", name="wb8")
            b8 = w_state["b8"]
            stg = stage_w.tile([P, KH], f32, tag="stgw", name="stgw")
            nc.scalar.dma_start(stg[:], ws[jb * P:(jb + 1) * P, h * KH:(h + 1) * KH])
            bin_w(b8[:, h * KH:(h + 1) * KH], stg[:])
            if h == 1:
                jc, sub = divmod(jb, JB_PER_JC)
                queue_tgroups(b8, wbT[jc], sub * P, "w")
                w_state["jb"] += 1
                w_state["half"] = 0
            else:
                w_state["half"] = 1


        def mm_block(ib, jc, pump_between=False):
            ps = psum_mm.tile([P, JBLK], f32, tag="ps", name="ps")
            if use_dr:
                nk = KT // 2
                for kp in range(nk):
                    nc.tensor.matmul(
                        ps[:],
                        lhsT=xbT[ib][:, 2 * kp:2 * kp + 2, :],
                        rhs=wbT[jc][:, 2 * kp:2 * kp + 2, :],
                        start=(kp == 0), stop=(kp == nk - 1),
                        perf_mode=mybir.MatmulPerfMode.DoubleRow,
                    )
                    if pump_between:
                        pump(1)
            else:
                for kt in range(KT):
                    nc.tensor.matmul(
                        ps[:],
                        lhsT=xbT[ib][:, kt, :],
                        rhs=wbT[jc][:, kt, :],
                        start=(kt == 0), stop=(kt == KT - 1),
                    )
                    if pump_between:
                        pump(1)
            ob = out_pool.tile([P, JBLK], f32, tag="ob", name="ob")
            # products are +/-0.5 (x) * +/-1 (w) = +/-0.5 -> scale by 2
            nc.any.tensor_scalar_mul(ob[:], ps[:], 2.0)
            nc.sync.dma_start(
                out[ib * P:(ib + 1) * P, jc * JBLK:(jc + 1) * JBLK], ob[:]
            )

        if interleave:
            # Startup: the first j-chunk of w plus the first two x blocks. Pad
            # the PE stream with warmup matmuls so the HAM window never sees
            # idle while the startup DMAs land.
            startup = [prep_w] * JB_PER_JC + [prep_x] * 2
            for i, prep in enumerate(startup):
                prep(i if prep is prep_w else i - JB_PER_JC)
                while pending:
                    pump(1)
                    if warmup:
                        warm(4)
                if warmup:
                    warm(16)
            # Pass jc over all i blocks; x-prep runs two iterations ahead
            # (the load->binarize->transpose chain is ~one iteration deep) and
            # the NEXT pass's w chunk is spread across this pass, one block
            # every IB//4 iterations, keeping input DMA flowing all schedule.
            for jc in range(JC):
                for ib in range(IB):
                    if jc == 0 and ib + 2 < IB:
                        prep_x(ib + 2)
                    if jc < JC - 1 and ib % 2 == 0:
                        prep_w_half()
                    mm_block(ib, jc, pump_between=True)
            while w_state["jb"] < JB:
                prep_w_half()
            pump(len(pending))
        else:
            for jb in range(JB):
                prep_w(jb)
                pump(len(pending))
            for ib in range(IB):
                prep_x(ib)
                pump(len(pending))
                mm_block(ib, 0)
            for jc in range(1, JC):
                for ib in range(IB):
                    mm_block(ib, jc)

    nc.compile()
    _PROGRAM_CACHE[key] = nc
    return nc


def kernel(x, weight):
    x = np.ascontiguousarray(np.asarray(x), dtype=np.float32)
    w = np.ascontiguousarray(np.asarray(weight), dtype=np.float32)
    assert x.shape == (FULL_M, FULL_K) and w.shape == (FULL_N, FULL_K)

    from concourse.bass_utils import run_bass_kernel_spmd

    nc = build_program()
    in_maps = []
    for c in range(N_CORES):
        r, s = divmod(c, GRID_J)
        in_maps.append({
            "xs": x[r * M_SH:(r + 1) * M_SH],
            "ws": w[s * N_SH:(s + 1) * N_SH],
        })
    res = run_bass_kernel_spmd(nc, in_maps, core_ids=list(range(N_CORES))).results
    outp = np.empty((FULL_M, FULL_N), dtype=np.float32)
    for c in range(N_CORES):
        r, s = divmod(c, GRID_J)
        outp[r * M_SH:(r + 1) * M_SH, s * N_SH:(s + 1) * N_SH] = res[c]["out"]
    return outp



# revision 19
# speedup vs baseline: 1.8758x; 1.0623x over previous
"""Binarized linear: out = sign(x+eps) @ sign(w+eps).T on 8 trn2 cores.

Sharding: 4x2 grid. Core c=(r,s): rows x[r*2048:(r+1)*2048], rows w[s*2048:(s+1)*2048].
Each core computes a [2048, 2048] output block; host concatenates. No collectives.

Per-core kernel (all arithmetic exact -> rel err 0 vs the f32 reference):
  - binarize x-shard to fp8e4m3 as +/-0.5 (DVE: (x>=0)-0.5), w-shard as +/-1
    (ACT Sign with +1e-20 bias, matching sign(v+1e-20))
  - transpose both to [K-on-partition] layout via plain fp8 matmul against an
    identity matrix (PE; counts as PE-busy so the HAM clock gate stays warm)
  - fp8 DoubleRow matmuls accumulate K=256 per instruction into fp32 PSUM
    (products +/-0.5, sums exact multiples of 0.5 well under 2^24)
  - evict scales by 2 -> exact integer outputs

Schedule: transpose work is chopped into 4-matmul groups and pumped between
DoubleRow matmuls so the PE never idles; x-preps run two iterations ahead of
their matmul group; later w chunks stream in during earlier passes; warmup
matmuls on the identity pad the startup so the clock gate opens early.
"""

from collections import deque

import numpy as np

P = 128
GRID_I, GRID_J = 4, 2
N_CORES = 8
FULL_M, FULL_N, FULL_K = 8192, 4096, 4096
M_SH, N_SH = FULL_M // GRID_I, FULL_N // GRID_J  # 2048, 2048

_PROGRAM_CACHE = {}


def build_program(m_sh=M_SH, n_sh=N_SH, k=FULL_K, use_dr=True, warmup=64,
                  interleave=True):
    """Build (and cache) the per-core Bass program. Same SPMD program on all cores."""
    key = (m_sh, n_sh, k, use_dr, warmup, interleave)
    if key in _PROGRAM_CACHE:
        return _PROGRAM_CACHE[key]

    from contextlib import ExitStack

    import concourse.bass as bass
    import concourse.mybir as mybir
    from concourse import bacc, tile
    from concourse.masks import make_identity

    f32 = mybir.dt.float32
    fp8 = mybir.dt.float8e4

    KT = k // P          # number of 128-wide k tiles
    KH = k // 2          # half-row staging width
    IB = m_sh // P       # i blocks (x rows / 128)
    JB = n_sh // P       # j blocks (w rows / 128)
    JBLK = 512           # j chunk width (matmul free dim)
    JC = n_sh // JBLK    # j chunks
    JB_PER_JC = JBLK // P
    KG = 4               # k-tiles per transpose-evict group
    assert KT % KG == 0 and KT % 2 == 0

    nc = bacc.Bacc("TRN2", target_bir_lowering=False, debug=False)
    xs = nc.dram_tensor("xs", [m_sh, k], f32, kind="ExternalInput").ap()
    ws = nc.dram_tensor("ws", [n_sh, k], f32, kind="ExternalInput").ap()
    out = nc.dram_tensor("out", [m_sh, n_sh], f32, kind="ExternalOutput").ap()

    with tile.TileContext(nc) as tc, ExitStack() as ctx:
        const_pool = ctx.enter_context(tc.tile_pool(name="const", bufs=1))
        stage_x = ctx.enter_context(tc.tile_pool(name="stagex", bufs=4))
        stage_w = ctx.enter_context(tc.tile_pool(name="stagew", bufs=2))
        b8_pool = ctx.enter_context(tc.tile_pool(name="b8", bufs=3))
        xbt_pool = ctx.enter_context(tc.tile_pool(name="xbt", bufs=1))
        wbt_pool = ctx.enter_context(tc.tile_pool(name="wbt", bufs=1))
        out_pool = ctx.enter_context(tc.tile_pool(name="outp", bufs=3))
        psum_t = ctx.enter_context(tc.tile_pool(name="pst", bufs=3, space="PSUM"))
        psum_mm = ctx.enter_context(tc.tile_pool(name="psmm", bufs=4, space="PSUM"))

        ident = const_pool.tile([P, P], fp8, tag="ident")
        make_identity(nc, ident)
        sign_bias = const_pool.tile([P, 1], f32, tag="sbias")
        nc.any.memset(sign_bias[:], 1e-20)

        # PE warmup: matmuls with no data dependency so the HAM clock-gate
        # opens to 8/8 while the first input DMAs are still in flight.
        psum_wu = ctx.enter_context(tc.tile_pool(name="pswu", bufs=1, space="PSUM"))
        warm_psum = psum_wu.tile([P, P], f32, tag="warm", name="warm") if warmup else None

        def warm(n):
            for _ in range(n):
                nc.tensor.matmul(warm_psum[:], lhsT=ident[:], rhs=ident[:],
                                 start=True, stop=True)

        if warmup:
            warm(warmup)

        # Resident transposed binarized operands, K on partitions:
        #   xbT[ib][kp, kt, i] = bin(xs[ib*128 + i, kt*128 + kp])   (+/-0.5)
        #   wbT[jc][kp, kt, j] = bin(ws[jc*512 + j, kt*128 + kp])   (+/-1)
        xbT = [
            xbt_pool.tile([P, KT, P], fp8, tag=f"xbt{ib}", name=f"xbt{ib}")
            for ib in range(IB)
        ]
        wbT = [
            wbt_pool.tile([P, KT, JBLK], fp8, tag=f"wbt{jc}", name=f"wbt{jc}")
            for jc in range(JC)
        ]

        def bin_x(b8h, stgh):
            # (v >= 0) -> {1,0}; minus 0.5 -> +/-0.5. Matches sign(v+1e-20) up
            # to the measure-zero region (-1e-20, 0) that f32 randn never hits.
            nc.vector.tensor_scalar(
                b8h, stgh, 0.0, 0.5,
                mybir.AluOpType.is_ge, mybir.AluOpType.subtract,
            )

        def bin_w(b8h, stgh):
            nc.scalar.sign(b8h, stgh, bias=sign_bias[:])  # sign(w+1e-20) -> +/-1

        def load_binarize(src_rows, pool, stg_tag, b8_tag, binarize):
            """Load 128 rows x k f32 (two half DMAs), binarize to fp8."""
            b8 = b8_pool.tile([P, k], fp8, tag=b8_tag, name=b8_tag)
            for h in range(2):
                stg = pool.tile([P, KH], f32, tag=stg_tag, name=stg_tag)
                nc.sync.dma_start(stg[:], src_rows[:, h * KH:(h + 1) * KH])
                binarize(b8[:, h * KH:(h + 1) * KH], stg[:])
            return b8

        pending = deque()  # transpose-group closures (each ~4 PE matmuls)

        def queue_tgroups(b8, dest, dest_col0, which):
            for kg in range(KT // KG):
                def g(kg=kg, b8=b8, dest=dest, dest_col0=dest_col0):
                    pt = psum_t.tile([P, KG, P], f32, tag="pt", name="pt")
                    for t in range(KG):
                        kt = kg * KG + t
                        nc.tensor.matmul(
                            pt[:, t, :],
                            lhsT=b8[:, kt * P:(kt + 1) * P],
                            rhs=ident[:],
                            start=True, stop=True,
                        )
                    nc.any.tensor_copy(
                        dest[:, kg * KG:(kg + 1) * KG, dest_col0:dest_col0 + P],
                        pt[:],
                    )
                pending.append(g)

        def pump(n):
            for _ in range(n):
                if not pending:
                    return
                pending.popleft()()

        def prep_x(ib):
            b8 = load_binarize(xs[ib * P:(ib + 1) * P, :], stage_x, "stgx", "xb8",
                               bin_x)
            queue_tgroups(b8, xbT[ib], 0, "x")

        def prep_w(jb):
            jc, sub = divmod(jb, JB_PER_JC)
            b8 = load_binarize(ws[jb * P:(jb + 1) * P, :], stage_w, "stgw", "wb8",
                               bin_w)
            queue_tgroups(b8, wbT[jc], sub * P, "w")

        # Later w chunks stream in at half-block granularity (one 1MB load +
        # one Sign every other iteration) so their DMA/ACT bursts never
        # displace the critical x-prep chain.
        w_state = {"jb": JB_PER_JC, "half": 0, "b8": None}

        def prep_w_half():
            jb = w_state["jb"]
            if jb >= JB:
                return
            h = w_state["half"]
            if h == 0:
                w_state["b8"] = b8_pool.tile([P, k], fp8, tag="wb8", name="wb8")
            b8 = w_state["b8"]
            stg = stage_w.tile([P, KH], f32, tag="stgw", name="stgw")
            nc.sync.dma_start(stg[:], ws[jb * P:(jb + 1) * P, h * KH:(h + 1) * KH])
            bin_w(b8[:, h * KH:(h + 1) * KH], stg[:])
            if h == 1:
                jc, sub = divmod(jb, JB_PER_JC)
                queue_tgroups(b8, wbT[jc], sub * P, "w")
                w_state["jb"] += 1
                w_state["half"] = 0
            else:
                w_state["half"] = 1


        def mm_block(ib, jc, pump_between=False):
            ps = psum_mm.tile([P, JBLK], f32, tag="ps", name="ps")
            if use_dr:
                nk = KT // 2
                for kp in range(nk):
                    nc.tensor.matmul(
                        ps[:],
                        lhsT=xbT[ib][:, 2 * kp:2 * kp + 2, :],
                        rhs=wbT[jc][:, 2 * kp:2 * kp + 2, :],
                        start=(kp == 0), stop=(kp == nk - 1),
                        perf_mode=mybir.MatmulPerfMode.DoubleRow,
                    )
                    if pump_between:
                        pump(1)
            else:
                for kt in range(KT):
                    nc.tensor.matmul(
                        ps[:],
                        lhsT=xbT[ib][:, kt, :],
                        rhs=wbT[jc][:, kt, :],
                        start=(kt == 0), stop=(kt == KT - 1),
                    )
                    if pump_between:
                        pump(1)
            ob = out_pool.tile([P, JBLK], f32, tag="ob", name="ob")
            # products are +/-0.5 (x) * +/-1 (w) = +/-0.5 -> scale by 2
            nc.any.tensor_scalar_mul(ob[:], ps[:], 2.0)
            nc.sync.dma_start(
                out[ib * P:(ib + 1) * P, jc * JBLK:(jc + 1) * JBLK], ob[:]
            )

        if interleave:
            # Startup: the first j-chunk of w plus the first two x blocks. Pad
            # the PE stream with warmup matmuls so the HAM window never sees
            # idle while the startup DMAs land.
            startup = [prep_w] * JB_PER_JC + [prep_x] * 2
            for i, prep in enumerate(startup):
                prep(i if prep is prep_w else i - JB_PER_JC)
                while pending:
                    pump(1)
                    if warmup:
                        warm(4)
                if warmup:
                    warm(16)
            # Pass jc over all i blocks; x-prep runs two iterations ahead
            # (the load->binarize->transpose chain is ~one iteration deep) and
            # the NEXT pass's w chunk is spread across this pass, one block
            # every IB//4 iterations, keeping input DMA flowing all schedule.
            for jc in range(JC):
                for ib in range(IB):
                    if jc == 0 and ib + 2 < IB:
                        prep_x(ib + 2)
                    if jc < JC - 1 and ib % 2 == 0:
                        prep_w_half()
                    mm_block(ib, jc, pump_between=True)
            while w_state["jb"] < JB:
                prep_w_half()
            pump(len(pending))
        else:
            for jb in range(JB):
                prep_w(jb)
                pump(len(pending))
            for ib in range(IB):
                prep_x(ib)
                pump(len(pending))
                mm_block(ib, 0)
            for jc in range(1, JC):
                for ib in range(IB):
                    mm_block(ib, jc)

    nc.compile()
    _PROGRAM_CACHE[key] = nc
    return nc


def kernel(x, weight):
    x = np.ascontiguousarray(np.asarray(x), dtype=np.float32)
    w = np.ascontiguousarray(np.asarray(weight), dtype=np.float32)
    assert x.shape == (FULL_M, FULL_K) and w.shape == (FULL_N, FULL_K)

    from concourse.bass_utils import run_bass_kernel_spmd

    nc = build_program()
    in_maps = []
    for c in range(N_CORES):
        r, s = divmod(c, GRID_J)
        in_maps.append({
            "xs": x[r * M_SH:(r + 1) * M_SH],
            "ws": w[s * N_SH:(s + 1) * N_SH],
        })
    res = run_bass_kernel_spmd(nc, in_maps, core_ids=list(range(N_CORES))).results
    outp = np.empty((FULL_M, FULL_N), dtype=np.float32)
    for c in range(N_CORES):
        r, s = divmod(c, GRID_J)
        outp[r * M_SH:(r + 1) * M_SH, s * N_SH:(s + 1) * N_SH] = res[c]["out"]
    return outp



# revision 21
# speedup vs baseline: 1.8893x; 1.0072x over previous
"""Binarized linear: out = sign(x+eps) @ sign(w+eps).T on 8 trn2 cores.

Sharding: 4x2 grid. Core c=(r,s): rows x[r*2048:(r+1)*2048], rows w[s*2048:(s+1)*2048].
Each core computes a [2048, 2048] output block; host concatenates. No collectives.

Per-core kernel (all arithmetic exact -> rel err 0 vs the f32 reference):
  - binarize x-shard to fp8e4m3 as +/-0.5 (DVE: (x>=0)-0.5), w-shard as +/-1
    (ACT Sign with +1e-20 bias, matching sign(v+1e-20))
  - transpose both to [K-on-partition] layout via plain fp8 matmul against an
    identity matrix (PE; counts as PE-busy so the HAM clock gate stays warm)
  - fp8 DoubleRow matmuls accumulate K=256 per instruction into fp32 PSUM
    (products +/-0.5, sums exact multiples of 0.5 well under 2^24)
  - evict scales by 2 -> exact integer outputs

Schedule: transpose work is chopped into 4-matmul groups and pumped between
DoubleRow matmuls so the PE never idles; x-preps run two iterations ahead of
their matmul group; later w chunks stream in during earlier passes; warmup
matmuls on the identity pad the startup so the clock gate opens early.
"""

from collections import deque

import numpy as np

P = 128
GRID_I, GRID_J = 4, 2
N_CORES = 8
FULL_M, FULL_N, FULL_K = 8192, 4096, 4096
M_SH, N_SH = FULL_M // GRID_I, FULL_N // GRID_J  # 2048, 2048

_PROGRAM_CACHE = {}


def build_program(m_sh=M_SH, n_sh=N_SH, k=FULL_K, use_dr=True, warmup=64,
                  interleave=True):
    """Build (and cache) the per-core Bass program. Same SPMD program on all cores."""
    key = (m_sh, n_sh, k, use_dr, warmup, interleave)
    if key in _PROGRAM_CACHE:
        return _PROGRAM_CACHE[key]

    from contextlib import ExitStack

    import concourse.bass as bass
    import concourse.mybir as mybir
    from concourse import bacc, tile
    from concourse.masks import make_identity

    f32 = mybir.dt.float32
    fp8 = mybir.dt.float8e4

    KT = k // P          # number of 128-wide k tiles
    KH = k // 2          # half-row staging width
    IB = m_sh // P       # i blocks (x rows / 128)
    JB = n_sh // P       # j blocks (w rows / 128)
    JBLK = 512           # j chunk width (matmul free dim)
    JC = n_sh // JBLK    # j chunks
    JB_PER_JC = JBLK // P
    KG = 4               # k-tiles per transpose-evict group
    assert KT % KG == 0 and KT % 2 == 0

    nc = bacc.Bacc("TRN2", target_bir_lowering=False, debug=False)
    xs = nc.dram_tensor("xs", [m_sh, k], f32, kind="ExternalInput").ap()
    ws = nc.dram_tensor("ws", [n_sh, k], f32, kind="ExternalInput").ap()
    out = nc.dram_tensor("out", [m_sh, n_sh], f32, kind="ExternalOutput").ap()

    with tile.TileContext(nc) as tc, ExitStack() as ctx:
        const_pool = ctx.enter_context(tc.tile_pool(name="const", bufs=1))
        stage_x = ctx.enter_context(tc.tile_pool(name="stagex", bufs=4))
        stage_w = ctx.enter_context(tc.tile_pool(name="stagew", bufs=2))
        b8_pool = ctx.enter_context(tc.tile_pool(name="b8", bufs=3))
        xbt_pool = ctx.enter_context(tc.tile_pool(name="xbt", bufs=1))
        wbt_pool = ctx.enter_context(tc.tile_pool(name="wbt", bufs=1))
        out_pool = ctx.enter_context(tc.tile_pool(name="outp", bufs=3))
        psum_t = ctx.enter_context(tc.tile_pool(name="pst", bufs=4, space="PSUM"))
        psum_mm = ctx.enter_context(tc.tile_pool(name="psmm", bufs=3, space="PSUM"))

        ident = const_pool.tile([P, P], fp8, tag="ident")
        make_identity(nc, ident)
        sign_bias = const_pool.tile([P, 1], f32, tag="sbias")
        nc.any.memset(sign_bias[:], 1e-20)

        # PE warmup: matmuls with no data dependency so the HAM clock-gate
        # opens to 8/8 while the first input DMAs are still in flight.
        psum_wu = ctx.enter_context(tc.tile_pool(name="pswu", bufs=1, space="PSUM"))
        warm_psum = psum_wu.tile([P, P], f32, tag="warm", name="warm") if warmup else None

        def warm(n):
            for _ in range(n):
                nc.tensor.matmul(warm_psum[:], lhsT=ident[:], rhs=ident[:],
                                 start=True, stop=True)

        if warmup:
            warm(warmup)

        # Resident transposed binarized operands, K on partitions:
        #   xbT[ib][kp, kt, i] = bin(xs[ib*128 + i, kt*128 + kp])   (+/-0.5)
        #   wbT[jc][kp, kt, j] = bin(ws[jc*512 + j, kt*128 + kp])   (+/-1)
        xbT = [
            xbt_pool.tile([P, KT, P], fp8, tag=f"xbt{ib}", name=f"xbt{ib}")
            for ib in range(IB)
        ]
        wbT = [
            wbt_pool.tile([P, KT, JBLK], fp8, tag=f"wbt{jc}", name=f"wbt{jc}")
            for jc in range(JC)
        ]

        def bin_x(b8h, stgh):
            # (v >= 0) -> {1,0}; minus 0.5 -> +/-0.5. Matches sign(v+1e-20) up
            # to the measure-zero region (-1e-20, 0) that f32 randn never hits.
            nc.vector.tensor_scalar(
                b8h, stgh, 0.0, 0.5,
                mybir.AluOpType.is_ge, mybir.AluOpType.subtract,
            )

        def bin_w(b8h, stgh):
            nc.scalar.sign(b8h, stgh, bias=sign_bias[:])  # sign(w+1e-20) -> +/-1

        def load_binarize(src_rows, pool, stg_tag, b8_tag, binarize):
            """Load 128 rows x k f32 (two half DMAs), binarize to fp8."""
            b8 = b8_pool.tile([P, k], fp8, tag=b8_tag, name=b8_tag)
            for h in range(2):
                stg = pool.tile([P, KH], f32, tag=stg_tag, name=stg_tag)
                nc.sync.dma_start(stg[:], src_rows[:, h * KH:(h + 1) * KH])
                binarize(b8[:, h * KH:(h + 1) * KH], stg[:])
            return b8

        pending = deque()  # transpose-group closures (each ~4 PE matmuls)

        def queue_tgroups(b8, dest, dest_col0, which):
            for kg in range(KT // KG):
                def g(kg=kg, b8=b8, dest=dest, dest_col0=dest_col0):
                    pt = psum_t.tile([P, KG, P], f32, tag="pt", name="pt")
                    for t in range(KG):
                        kt = kg * KG + t
                        nc.tensor.matmul(
                            pt[:, t, :],
                            lhsT=b8[:, kt * P:(kt + 1) * P],
                            rhs=ident[:],
                            start=True, stop=True,
                        )
                    nc.any.tensor_copy(
                        dest[:, kg * KG:(kg + 1) * KG, dest_col0:dest_col0 + P],
                        pt[:],
                    )
                pending.append(g)

        def pump(n):
            for _ in range(n):
                if not pending:
                    return
                pending.popleft()()

        def prep_x(ib):
            b8 = load_binarize(xs[ib * P:(ib + 1) * P, :], stage_x, "stgx", "xb8",
                               bin_x)
            queue_tgroups(b8, xbT[ib], 0, "x")

        def prep_w(jb):
            jc, sub = divmod(jb, JB_PER_JC)
            b8 = load_binarize(ws[jb * P:(jb + 1) * P, :], stage_w, "stgw", "wb8",
                               bin_w)
            queue_tgroups(b8, wbT[jc], sub * P, "w")

        # Later w chunks stream in at half-block granularity (one 1MB load +
        # one Sign every other iteration) so their DMA/ACT bursts never
        # displace the critical x-prep chain.
        w_state = {"jb": JB_PER_JC, "half": 0, "b8": None}

        def prep_w_half():
            jb = w_state["jb"]
            if jb >= JB:
                return
            h = w_state["half"]
            if h == 0:
                w_state["b8"] = b8_pool.tile([P, k], fp8, tag="wb8", name="wb8")
            b8 = w_state["b8"]
            stg = stage_w.tile([P, KH], f32, tag="stgw", name="stgw")
            nc.sync.dma_start(stg[:], ws[jb * P:(jb + 1) * P, h * KH:(h + 1) * KH])
            bin_w(b8[:, h * KH:(h + 1) * KH], stg[:])
            if h == 1:
                jc, sub = divmod(jb, JB_PER_JC)
                queue_tgroups(b8, wbT[jc], sub * P, "w")
                w_state["jb"] += 1
                w_state["half"] = 0
            else:
                w_state["half"] = 1


        def mm_block(ib, jc, pump_between=False):
            ps = psum_mm.tile([P, JBLK], f32, tag="ps", name="ps")
            if use_dr:
                nk = KT // 2
                for kp in range(nk):
                    nc.tensor.matmul(
                        ps[:],
                        lhsT=xbT[ib][:, 2 * kp:2 * kp + 2, :],
                        rhs=wbT[jc][:, 2 * kp:2 * kp + 2, :],
                        start=(kp == 0), stop=(kp == nk - 1),
                        perf_mode=mybir.MatmulPerfMode.DoubleRow,
                    )
                    if pump_between:
                        pump(1)
            else:
                for kt in range(KT):
                    nc.tensor.matmul(
                        ps[:],
                        lhsT=xbT[ib][:, kt, :],
                        rhs=wbT[jc][:, kt, :],
                        start=(kt == 0), stop=(kt == KT - 1),
                    )
                    if pump_between:
                        pump(1)
            ob = out_pool.tile([P, JBLK], f32, tag="ob", name="ob")
            # products are +/-0.5 (x) * +/-1 (w) = +/-0.5 -> scale by 2
            nc.any.tensor_scalar_mul(ob[:], ps[:], 2.0)
            nc.sync.dma_start(
                out[ib * P:(ib + 1) * P, jc * JBLK:(jc + 1) * JBLK], ob[:]
            )

        if interleave:
            # Startup: the first j-chunk of w plus the first two x blocks. Pad
            # the PE stream with warmup matmuls so the HAM window never sees
            # idle while the startup DMAs land.
            startup = [prep_w] * JB_PER_JC + [prep_x] * 2
            for i, prep in enumerate(startup):
                prep(i if prep is prep_w else i - JB_PER_JC)
                while pending:
                    pump(1)
                    if warmup:
                        warm(4)
                if warmup:
                    warm(16)
            # Queue x2/x3 as well (groups pump during pass 0), so pass 0
            # consumes x blocks with a 4-deep prefetch instead of 2: the
            # load->binarize->transpose chain latency stays hidden.
            prep_x(2)
            prep_x(3)
            # Pass jc over all i blocks; x-prep runs four iterations ahead and
            # the NEXT pass's w chunk is spread across this pass, one block
            # every IB//4 iterations, keeping input DMA flowing all schedule.
            for jc in range(JC):
                for ib in range(IB):
                    if jc == 0 and ib + 4 < IB:
                        prep_x(ib + 4)
                    if jc < JC - 1 and ib % 2 == 0:
                        prep_w_half()
                    mm_block(ib, jc, pump_between=True)
            while w_state["jb"] < JB:
                prep_w_half()
            pump(len(pending))
        else:
            for jb in range(JB):
                prep_w(jb)
                pump(len(pending))
            for ib in range(IB):
                prep_x(ib)
                pump(len(pending))
                mm_block(ib, 0)
            for jc in range(1, JC):
                for ib in range(IB):
                    mm_block(ib, jc)

    nc.compile()
    _PROGRAM_CACHE[key] = nc
    return nc


def kernel(x, weight):
    x = np.ascontiguousarray(np.asarray(x), dtype=np.float32)
    w = np.ascontiguousarray(np.asarray(weight), dtype=np.float32)
    assert x.shape == (FULL_M, FULL_K) and w.shape == (FULL_N, FULL_K)

    from concourse.bass_utils import run_bass_kernel_spmd

    nc = build_program()
    in_maps = []
    for c in range(N_CORES):
        r, s = divmod(c, GRID_J)
        in_maps.append({
            "xs": x[r * M_SH:(r + 1) * M_SH],
            "ws": w[s * N_SH:(s + 1) * N_SH],
        })
    res = run_bass_kernel_spmd(nc, in_maps, core_ids=list(range(N_CORES))).results
    outp = np.empty((FULL_M, FULL_N), dtype=np.float32)
    for c in range(N_CORES):
        r, s = divmod(c, GRID_J)
        outp[r * M_SH:(r + 1) * M_SH, s * N_SH:(s + 1) * N_SH] = res[c]["out"]
    return outp



# revision 22
# speedup vs baseline: 1.9326x; 1.0229x over previous
"""Binarized linear: out = sign(x+eps) @ sign(w+eps).T on 8 trn2 cores.

Sharding: 4x2 grid. Core c=(r,s): rows x[r*2048:(r+1)*2048], rows w[s*2048:(s+1)*2048].
Each core computes a [2048, 2048] output block; host concatenates. No collectives.

Per-core kernel (all arithmetic exact -> rel err 0 vs the f32 reference):
  - binarize x-shard to fp8e4m3 as +/-0.5 (DVE: (x>=0)-0.5), w-shard as +/-1
    (ACT Sign with +1e-20 bias, matching sign(v+1e-20))
  - transpose both to [K-on-partition] layout via plain fp8 matmul against an
    identity matrix (PE; counts as PE-busy so the HAM clock gate stays warm)
  - fp8 DoubleRow matmuls accumulate K=256 per instruction into fp32 PSUM
    (products +/-0.5, sums exact multiples of 0.5 well under 2^24)
  - evict scales by 2 -> exact integer outputs

Schedule: transpose work is chopped into 4-matmul groups and pumped between
DoubleRow matmuls so the PE never idles; x-preps run two iterations ahead of
their matmul group; later w chunks stream in during earlier passes; warmup
matmuls on the identity pad the startup so the clock gate opens early.
"""

from collections import deque

import numpy as np

P = 128
GRID_I, GRID_J = 4, 2
N_CORES = 8
FULL_M, FULL_N, FULL_K = 8192, 4096, 4096
M_SH, N_SH = FULL_M // GRID_I, FULL_N // GRID_J  # 2048, 2048

_PROGRAM_CACHE = {}


def build_program(m_sh=M_SH, n_sh=N_SH, k=FULL_K, use_dr=True, warmup=64,
                  interleave=True):
    """Build (and cache) the per-core Bass program. Same SPMD program on all cores."""
    key = (m_sh, n_sh, k, use_dr, warmup, interleave)
    if key in _PROGRAM_CACHE:
        return _PROGRAM_CACHE[key]

    from contextlib import ExitStack

    import concourse.bass as bass
    import concourse.mybir as mybir
    from concourse import bacc, tile
    from concourse.masks import make_identity

    f32 = mybir.dt.float32
    fp8 = mybir.dt.float8e4

    KT = k // P          # number of 128-wide k tiles
    KH = k // 2          # half-row staging width
    IB = m_sh // P       # i blocks (x rows / 128)
    JB = n_sh // P       # j blocks (w rows / 128)
    JBLK = 512           # j chunk width (matmul free dim)
    JC = n_sh // JBLK    # j chunks
    JB_PER_JC = JBLK // P
    KG = 4               # k-tiles per transpose-evict group
    assert KT % KG == 0 and KT % 2 == 0

    nc = bacc.Bacc("TRN2", target_bir_lowering=False, debug=False)
    xs = nc.dram_tensor("xs", [m_sh, k], f32, kind="ExternalInput").ap()
    ws = nc.dram_tensor("ws", [n_sh, k], f32, kind="ExternalInput").ap()
    out = nc.dram_tensor("out", [m_sh, n_sh], f32, kind="ExternalOutput").ap()

    with tile.TileContext(nc) as tc, ExitStack() as ctx:
        const_pool = ctx.enter_context(tc.tile_pool(name="const", bufs=1))
        stage_x = ctx.enter_context(tc.tile_pool(name="stagex", bufs=4))
        stage_w = ctx.enter_context(tc.tile_pool(name="stagew", bufs=2))
        b8_pool = ctx.enter_context(tc.tile_pool(name="b8", bufs=3))
        xbt_pool = ctx.enter_context(tc.tile_pool(name="xbt", bufs=1))
        wbt_pool = ctx.enter_context(tc.tile_pool(name="wbt", bufs=1))
        out_pool = ctx.enter_context(tc.tile_pool(name="outp", bufs=3))
        psum_t = ctx.enter_context(tc.tile_pool(name="pst", bufs=4, space="PSUM"))
        psum_mm = ctx.enter_context(tc.tile_pool(name="psmm", bufs=3, space="PSUM"))

        ident = const_pool.tile([P, P], fp8, tag="ident")
        make_identity(nc, ident)
        sign_bias = const_pool.tile([P, 1], f32, tag="sbias")
        nc.any.memset(sign_bias[:], 1e-20)

        # PE warmup: matmuls with no data dependency so the HAM clock-gate
        # opens to 8/8 while the first input DMAs are still in flight.
        psum_wu = ctx.enter_context(tc.tile_pool(name="pswu", bufs=1, space="PSUM"))
        warm_psum = psum_wu.tile([P, P], f32, tag="warm", name="warm") if warmup else None

        def warm(n):
            for _ in range(n):
                nc.tensor.matmul(warm_psum[:], lhsT=ident[:], rhs=ident[:],
                                 start=True, stop=True)

        if warmup:
            warm(warmup)

        # Resident transposed binarized operands, K on partitions:
        #   xbT[ib][kp, kt, i] = bin(xs[ib*128 + i, kt*128 + kp])   (+/-0.5)
        #   wbT[jc][kp, kt, j] = bin(ws[jc*512 + j, kt*128 + kp])   (+/-1)
        xbT = [
            xbt_pool.tile([P, KT, P], fp8, tag=f"xbt{ib}", name=f"xbt{ib}")
            for ib in range(IB)
        ]
        wbT = [
            wbt_pool.tile([P, KT, JBLK], fp8, tag=f"wbt{jc}", name=f"wbt{jc}")
            for jc in range(JC)
        ]

        def bin_x(b8h, stgh):
            # (v >= 0) -> {1,0}; minus 0.5 -> +/-0.5. Matches sign(v+1e-20) up
            # to the measure-zero region (-1e-20, 0) that f32 randn never hits.
            nc.vector.tensor_scalar(
                b8h, stgh, 0.0, 0.5,
                mybir.AluOpType.is_ge, mybir.AluOpType.subtract,
            )

        def bin_w(b8h, stgh):
            nc.scalar.sign(b8h, stgh, bias=sign_bias[:])  # sign(w+1e-20) -> +/-1

        def load_binarize(src_rows, pool, stg_tag, b8_tag, binarize):
            """Load 128 rows x k f32 (two half DMAs), binarize to fp8."""
            b8 = b8_pool.tile([P, k], fp8, tag=b8_tag, name=b8_tag)
            for h in range(2):
                stg = pool.tile([P, KH], f32, tag=stg_tag, name=stg_tag)
                nc.sync.dma_start(stg[:], src_rows[:, h * KH:(h + 1) * KH])
                binarize(b8[:, h * KH:(h + 1) * KH], stg[:])
            return b8

        pending = deque()  # transpose-group closures (each ~4 PE matmuls)

        def queue_tgroups(b8, dest, dest_col0, which):
            for kg in range(KT // KG):
                def g(kg=kg, b8=b8, dest=dest, dest_col0=dest_col0):
                    pt = psum_t.tile([P, KG, P], f32, tag="pt", name="pt")
                    for t in range(KG):
                        kt = kg * KG + t
                        nc.tensor.matmul(
                            pt[:, t, :],
                            lhsT=b8[:, kt * P:(kt + 1) * P],
                            rhs=ident[:],
                            start=True, stop=True,
                        )
                    nc.any.tensor_copy(
                        dest[:, kg * KG:(kg + 1) * KG, dest_col0:dest_col0 + P],
                        pt[:],
                    )
                pending.append(g)

        def pump(n):
            for _ in range(n):
                if not pending:
                    return
                pending.popleft()()

        def prep_x(ib):
            b8 = load_binarize(xs[ib * P:(ib + 1) * P, :], stage_x, "stgx", "xb8",
                               bin_x)
            queue_tgroups(b8, xbT[ib], 0, "x")

        def prep_w(jb):
            jc, sub = divmod(jb, JB_PER_JC)
            b8 = load_binarize(ws[jb * P:(jb + 1) * P, :], stage_w, "stgw", "wb8",
                               bin_w)
            queue_tgroups(b8, wbT[jc], sub * P, "w")

        # Later w chunks stream in at half-block granularity (one 1MB load +
        # one Sign every other iteration) so their DMA/ACT bursts never
        # displace the critical x-prep chain.
        w_state = {"jb": JB_PER_JC, "half": 0, "b8": None}

        def prep_w_half():
            jb = w_state["jb"]
            if jb >= JB:
                return
            h = w_state["half"]
            if h == 0:
                w_state["b8"] = b8_pool.tile([P, k], fp8, tag="wb8", name="wb8")
            b8 = w_state["b8"]
            stg = stage_w.tile([P, KH], f32, tag="stgw", name="stgw")
            nc.sync.dma_start(stg[:], ws[jb * P:(jb + 1) * P, h * KH:(h + 1) * KH])
            bin_w(b8[:, h * KH:(h + 1) * KH], stg[:])
            if h == 1:
                jc, sub = divmod(jb, JB_PER_JC)
                queue_tgroups(b8, wbT[jc], sub * P, "w")
                w_state["jb"] += 1
                w_state["half"] = 0
            else:
                w_state["half"] = 1


        def mm_block(ib, jc, pump_between=False):
            ps = psum_mm.tile([P, JBLK], f32, tag="ps", name="ps")
            if use_dr:
                nk = KT // 2
                for kp in range(nk):
                    nc.tensor.matmul(
                        ps[:],
                        lhsT=xbT[ib][:, 2 * kp:2 * kp + 2, :],
                        rhs=wbT[jc][:, 2 * kp:2 * kp + 2, :],
                        start=(kp == 0), stop=(kp == nk - 1),
                        perf_mode=mybir.MatmulPerfMode.DoubleRow,
                    )
                    if pump_between:
                        pump(1)
            else:
                for kt in range(KT):
                    nc.tensor.matmul(
                        ps[:],
                        lhsT=xbT[ib][:, kt, :],
                        rhs=wbT[jc][:, kt, :],
                        start=(kt == 0), stop=(kt == KT - 1),
                    )
                    if pump_between:
                        pump(1)
            ob = out_pool.tile([P, JBLK], f32, tag="ob", name="ob")
            # products are +/-0.5 (x) * +/-1 (w) = +/-0.5 -> scale by 2
            nc.any.tensor_scalar_mul(ob[:], ps[:], 2.0)
            nc.sync.dma_start(
                out[ib * P:(ib + 1) * P, jc * JBLK:(jc + 1) * JBLK], ob[:]
            )

        if interleave:
            # Startup: the first j-chunk of w plus the first two x blocks. Pad
            # the PE stream with warmup matmuls so the HAM window never sees
            # idle while the startup DMAs land.
            startup = [prep_w] * JB_PER_JC + [prep_x] * 2
            for i, prep in enumerate(startup):
                prep(i if prep is prep_w else i - JB_PER_JC)
                while pending:
                    pump(1)
                    if warmup:
                        warm(4)
                if warmup:
                    warm(16)
            # Queue x2/x3 as well (groups pump during pass 0), so pass 0
            # consumes x blocks with a 4-deep prefetch instead of 2: the
            # load->binarize->transpose chain latency stays hidden.
            prep_x(2)
            prep_x(3)
            # Pass jc over all i blocks; x-prep runs four iterations ahead and
            # the NEXT pass's w chunk is spread across this pass, one block
            # every IB//4 iterations, keeping input DMA flowing all schedule.
            for jc in range(JC):
                for ib in range(IB):
                    if jc == 0 and ib + 4 < IB:
                        prep_x(ib + 4)
                    if jc < JC - 1 and ib % 2 == 0:
                        prep_w_half()
                    mm_block(ib, jc, pump_between=True)
                # drain the transpose backlog at the pass boundary: the next
                # pass's first blocks consume these groups, so waiting here
                # (as PE work) beats stalling there
                pump(len(pending))
            while w_state["jb"] < JB:
                prep_w_half()
            pump(len(pending))
        else:
            for jb in range(JB):
                prep_w(jb)
                pump(len(pending))
            for ib in range(IB):
                prep_x(ib)
                pump(len(pending))
                mm_block(ib, 0)
            for jc in range(1, JC):
                for ib in range(IB):
                    mm_block(ib, jc)

    nc.compile()
    _PROGRAM_CACHE[key] = nc
    return nc


def kernel(x, weight):
    x = np.ascontiguousarray(np.asarray(x), dtype=np.float32)
    w = np.ascontiguousarray(np.asarray(weight), dtype=np.float32)
    assert x.shape == (FULL_M, FULL_K) and w.shape == (FULL_N, FULL_K)

    from concourse.bass_utils import run_bass_kernel_spmd

    nc = build_program()
    in_maps = []
    for c in range(N_CORES):
        r, s = divmod(c, GRID_J)
        in_maps.append({
            "xs": x[r * M_SH:(r + 1) * M_SH],
            "ws": w[s * N_SH:(s + 1) * N_SH],
        })
    res = run_bass_kernel_spmd(nc, in_maps, core_ids=list(range(N_CORES))).results
    outp = np.empty((FULL_M, FULL_N), dtype=np.float32)
    for c in range(N_CORES):
        r, s = divmod(c, GRID_J)
        outp[r * M_SH:(r + 1) * M_SH, s * N_SH:(s + 1) * N_SH] = res[c]["out"]
    return outp

